# revision 1
# baseline (speedup 1.0000x reference)
"""Trainium2 Bass kernel for 3-layer per-task LoRA MLP.

Full-input contract: kernel(**inputs) takes the unsharded tensors and returns
the full [8, 1024, 1024] output. Internally the task axis (t=8) is sharded
across 8 NeuronCores (one task per core); base weights are replicated.

Per-core layout strategy:
  - activations live transposed in SBUF: h^T [feat(part), batch(free)]
  - base weights k0/k1 stream in natural [K, M] layout as the matmul
    stationary operand; moving operand is the transposed activation
  - LoRA: z^T = (scaling*d)^T-contraction matmul, then the rank-8 delta is
    one extra accumulating matmul into the same PSUM group as the base
  - final layer uses h2^T as the *stationary* operand and k2 as the moving
    operand, producing natural-layout [batch, feat] output directly
  - fp32 bits are bitcast to float32r at matmul sites => 1 cycle/row (4x
    over plain fp32) for N>=256
"""

import sys

if "/opt/trn_rl_repo" not in sys.path:
    sys.path.insert(0, "/opt/trn_rl_repo")

import numpy as np

T, B, D = 8, 1024, 1024
H1, H2, H3 = 2048, 2048, 1024
R = 8
SCALING = 2.0  # alpha/rank = 16/8
P = 128
NT = 512  # PSUM free-dim tile (fp32 one-bank limit)

_CACHE = {}


def _build(mm_mode="f32r"):
    import concourse.bass as bass
    import concourse.mybir as mybir
    from concourse import bacc
    from concourse.tile import TileContext
    from concourse.bass import ts
    from concourse.masks import make_identity

    f32 = mybir.dt.float32
    f32r = mybir.dt.float32r
    AF = mybir.ActivationFunctionType

    fmm = f32r if mm_mode == "f32r" else f32

    def mc(ap):
        return ap

    nc = bacc.Bacc(None, target_bir_lowering=False, name="lora_mlp")

    x = nc.dram_tensor("x", (B, D), f32, kind="ExternalInput")
    k0 = nc.dram_tensor("k0", (D, H1), fmm, kind="ExternalInput")
    b0 = nc.dram_tensor("b0", (H1,), f32, kind="ExternalInput")
    d0 = nc.dram_tensor("d0", (D, R), fmm, kind="ExternalInput")
    u0 = nc.dram_tensor("u0", (R, H1), fmm, kind="ExternalInput")
    k1 = nc.dram_tensor("k1", (H1, H2), fmm, kind="ExternalInput")
    b1 = nc.dram_tensor("b1", (H2,), f32, kind="ExternalInput")
    d1 = nc.dram_tensor("d1", (H1, R), fmm, kind="ExternalInput")
    u1 = nc.dram_tensor("u1", (R, H2), fmm, kind="ExternalInput")
    k2 = nc.dram_tensor("k2", (H2, H3), fmm, kind="ExternalInput")
    b2 = nc.dram_tensor("b2", (H3,), fmm, kind="ExternalInput")
    d2 = nc.dram_tensor("d2", (H2, R), fmm, kind="ExternalInput")
    u2 = nc.dram_tensor("u2", (R, H3), fmm, kind="ExternalInput")
    out = nc.dram_tensor("out", (B, H3), f32, kind="ExternalOutput")

    KT0 = D // P      # 8  k-tiles, layer 0
    KT1 = H1 // P     # 16 k-tiles, layer 1
    KT2 = H2 // P     # 16 k-tiles, layer 2
    MT0 = H1 // P     # 16 m-tiles, layer 0
    MT1 = H2 // P     # 16 m-tiles, layer 1
    BT = B // P       # 8  batch 128-tiles
    NB = B // NT      # 2  batch 512-halves (free dim, layers 0/1)
    N2 = H3 // NT     # 2  feature 512-halves (free dim, layer 2)
    KG2 = 4           # layer-2 k-group size (k2 streamed in groups)

    with TileContext(nc) as tc:
        with (
            tc.tile_pool(name="main", bufs=1) as pool,
            tc.tile_pool(name="psum", bufs=1, space="PSUM") as pp,
        ):
            ident = pool.tile([P, P], f32, tag="ident", bufs=1)
            make_identity(nc, ident)
            ones_f = pool.tile([1, P], f32, tag="ones_f", bufs=1)
            nc.vector.memset(ones_f, 1.0)
            ones = pool.tile([1, P], fmm, tag="ones", bufs=1)
            nc.vector.tensor_copy(ones, ones_f)

            # small constants: lora d (pre-scaled on host), u, biases
            d0_sb = pool.tile([P, KT0 * R], fmm, tag="d0", bufs=1)
            nc.sync.dma_start(
                out=d0_sb.rearrange("p (k r) -> p k r", r=R),
                in_=d0[:, :].rearrange("(k p) r -> p k r", p=P),
            )
            d1_sb = pool.tile([P, KT1 * R], fmm, tag="d1", bufs=1)
            nc.sync.dma_start(
                out=d1_sb.rearrange("p (k r) -> p k r", r=R),
                in_=d1[:, :].rearrange("(k p) r -> p k r", p=P),
            )
            d2_sb = pool.tile([P, KT2 * R], fmm, tag="d2", bufs=1)
            nc.sync.dma_start(
                out=d2_sb.rearrange("p (k r) -> p k r", r=R),
                in_=d2[:, :].rearrange("(k p) r -> p k r", p=P),
            )
            u0_sb = pool.tile([R, H1], fmm, tag="u", bufs=1)
            nc.sync.dma_start(out=u0_sb, in_=u0[:, :])

            b0_sb = pool.tile([P, MT0], f32, tag="b0", bufs=1)
            for m in range(MT0):
                nc.sync.dma_start(
                    out=b0_sb[:, ts(m, 1)], in_=b0[ts(m, P)].unsqueeze(1)
                )
            b1_sb = pool.tile([P, MT1], f32, tag="b1", bufs=1)
            for m in range(MT1):
                nc.sync.dma_start(
                    out=b1_sb[:, ts(m, 1)], in_=b1[ts(m, P)].unsqueeze(1)
                )
            b2_sb = pool.tile([1, H3], fmm, tag="b2", bufs=1)
            nc.sync.dma_start(out=b2_sb, in_=b2[:].unsqueeze(0))

            # ---- load x and transpose to xT [D(part), B(free)] ----
            xT = []
            for di in range(KT0):
                xT.append(pool.tile([P, B], fmm, tag="E", bufs=8, name=f"xT{di}"))
            for bi in range(BT):
                xn = pool.tile([P, D], f32, tag="xn", bufs=3)
                nc.sync.dma_start(out=xn, in_=x[ts(bi, P), :])
                for di in range(KT0):
                    pt = pp.tile([P, P], f32, tag="pt", bufs=2)
                    nc.tensor.transpose(pt, xn[:, ts(di, P)], ident)
                    nc.vector.tensor_copy(xT[di][:, ts(bi, P)], pt)

            def lora_zT(d_sb, kt, src_tiles, tag):
                """z^T [R, B] = (scaling*d)^T @ h  via PSUM accumulation."""
                z_sb = pool.tile([R, B], fmm, tag=tag, bufs=1)
                for n in range(NB):
                    pz = pp.tile([R, NT], f32, tag="pz", bufs=1)
                    for k in range(kt):
                        nc.tensor.matmul(
                            pz,
                            mc(d_sb[:, ts(k, R)]),
                            mc(src_tiles[k][:, ts(n, NT)]),
                            start=(k == 0),
                            stop=(k == kt - 1),
                        )
                    nc.scalar.copy(z_sb[:, ts(n, NT)], pz)
                return z_sb

            # =================== layer 0 ===================
            z0 = lora_zT(d0_sb, KT0, xT, "z")
            h0T = []
            for m in range(MT0):
                w = pool.tile([P, KT0 * P], fmm, tag="W", bufs=4)
                nc.sync.dma_start(
                    out=w.rearrange("p (k c) -> p k c", c=P),
                    in_=k0[:, ts(m, P)].rearrange("(k p) c -> p k c", p=P),
                )
                ht = pool.tile([P, B], fmm, tag="B", bufs=16)
                h0T.append(ht)
                for n in range(NB):
                    ps = pp.tile([P, NT], f32, tag="pm", bufs=5)
                    for k in range(KT0):
                        nc.tensor.matmul(
                            ps,
                            mc(w[:, ts(k, P)]),
                            mc(xT[k][:, ts(n, NT)]),
                            start=(k == 0),
                            stop=False,
                        )
                    nc.tensor.matmul(
                        ps,
                        mc(u0_sb[:, ts(m, P)]),
                        mc(z0[:, ts(n, NT)]),
                        start=False,
                        stop=True,
                    )
                    nc.scalar.activation(
                        ht[:, ts(n, NT)], ps, AF.Relu, bias=b0_sb[:, ts(m, 1)]
                    )

            # =================== layer 1 ===================
            u1_sb = pool.tile([R, H2], fmm, tag="u", bufs=1)
            nc.sync.dma_start(out=u1_sb, in_=u1[:, :])
            z1 = lora_zT(d1_sb, KT1, h0T, "z")
            h1T = []
            for m in range(MT1):
                wa = pool.tile([P, 8 * P], fmm, tag="W", bufs=4)
                nc.sync.dma_start(
                    out=wa.rearrange("p (k c) -> p k c", c=P),
                    in_=k1[0:1024, ts(m, P)].rearrange("(k p) c -> p k c", p=P),
                )
                wb = pool.tile([P, 8 * P], fmm, tag="W", bufs=4)
                nc.sync.dma_start(
                    out=wb.rearrange("p (k c) -> p k c", c=P),
                    in_=k1[1024:2048, ts(m, P)].rearrange("(k p) c -> p k c", p=P),
                )
                ht = pool.tile([P, B], fmm, tag="A", bufs=16)
                h1T.append(ht)
                for n in range(NB):
                    ps = pp.tile([P, NT], f32, tag="pm", bufs=5)
                    for k in range(KT1):
                        wsrc = wa if k < 8 else wb
                        nc.tensor.matmul(
                            ps,
                            mc(wsrc[:, ts(k % 8, P)]),
                            mc(h0T[k][:, ts(n, NT)]),
                            start=(k == 0),
                            stop=False,
                        )
                    nc.tensor.matmul(
                        ps,
                        mc(u1_sb[:, ts(m, P)]),
                        mc(z1[:, ts(n, NT)]),
                        start=False,
                        stop=True,
                    )
                    nc.scalar.activation(
                        ht[:, ts(n, NT)], ps, AF.Relu, bias=b1_sb[:, ts(m, 1)]
                    )

            # =================== layer 2 (natural output) ===================
            u2_sb = pool.tile([R, H3], fmm, tag="u", bufs=1)
            nc.sync.dma_start(out=u2_sb, in_=u2[:, :])
            z2 = lora_zT(d2_sb, KT2, h1T, "z")
            out_acc = [None] * BT
            for g in range(KT2 // KG2):
                kg = []
                for j in range(KG2):
                    kt_ = pool.tile([P, H3], fmm, tag="E", bufs=8)
                    kg.append(kt_)
                    nc.sync.dma_start(out=kt_, in_=k2[ts(g * KG2 + j, P), :])
                for m in range(BT):
                    if g == 0:
                        out_acc[m] = pool.tile([P, H3], f32, tag="B", bufs=16, name=f"oacc{m}")
                    for n in range(N2):
                        ps = pp.tile([P, NT], f32, tag="pm", bufs=5)
                        first = True
                        if g == 0:
                            # bias broadcast over partitions: b2[m,n] += b2[n]
                            nc.tensor.matmul(
                                ps,
                                mc(ones),
                                mc(b2_sb[:, ts(n, NT)]),
                                start=True,
                                stop=False,
                            )
                            first = False
                        is_last = g == KT2 // KG2 - 1
                        for j in range(KG2):
                            k = g * KG2 + j
                            nc.tensor.matmul(
                                ps,
                                mc(h1T[k][:, ts(m, P)]),
                                mc(kg[j][:, ts(n, NT)]),
                                start=first,
                                stop=(not is_last) and j == KG2 - 1,
                            )
                            first = False
                        if is_last:
                            # rank-8 LoRA delta folded into the same PSUM group
                            nc.tensor.matmul(
                                ps,
                                mc(z2[:, ts(m, P)]),
                                mc(u2_sb[:, ts(n, NT)]),
                                start=False,
                                stop=True,
                            )
                        if g == 0:
                            nc.vector.tensor_copy(out_acc[m][:, ts(n, NT)], ps)
                        else:
                            nc.vector.tensor_add(
                                out_acc[m][:, ts(n, NT)],
                                out_acc[m][:, ts(n, NT)],
                                ps,
                            )
                for m in range(BT):
                    if g == KT2 // KG2 - 1:
                        nc.sync.dma_start(out=out[ts(m, P), :], in_=out_acc[m])

    if not nc.is_finalized():
        nc.finalize()
    return nc


def _get_nc():
    if "nc" not in _CACHE:
        _CACHE["nc"] = _build()
    return _CACHE["nc"]


def build_in_maps(inputs):
    def c(a):
        return np.ascontiguousarray(a, dtype=np.float32)

    in_maps = []
    for t in range(T):
        in_maps.append(
            {
                "x": c(inputs["x"][t]),
                "k0": c(inputs["k0"]),
                "b0": c(inputs["b0"]),
                "d0": c(inputs["d0"][:, :, t] * SCALING),
                "u0": c(inputs["u0"][:, :, t]),
                "k1": c(inputs["k1"]),
                "b1": c(inputs["b1"]),
                "d1": c(inputs["d1"][:, :, t] * SCALING),
                "u1": c(inputs["u1"][:, :, t]),
                "k2": c(inputs["k2"]),
                "b2": c(inputs["b2"]),
                "d2": c(inputs["d2"][:, :, t] * SCALING),
                "u2": c(inputs["u2"][:, :, t]),
            }
        )
    return in_maps


def kernel(**inputs):
    from concourse import bass_utils

    nc = _get_nc()
    in_maps = build_in_maps(inputs)
    res = bass_utils.run_bass_kernel_spmd(nc, in_maps, core_ids=list(range(T)))
    return np.stack([r["out"] for r in res.results], axis=0)



# revision 2
# speedup vs baseline: 4.1327x; 4.1327x over previous
"""Trainium2 Bass kernel for 3-layer per-task LoRA MLP.

Full-input contract: kernel(**inputs) takes the unsharded tensors and returns
the full [8, 1024, 1024] output. Internally the task axis (t=8) is sharded
across 8 NeuronCores (one task per core).

Strategy:
  - LoRA is folded on the host into per-task effective weights
    W_eff = k + (alpha/r) * d @ u  (standard LoRA weight merging), so the
    device kernel is a plain 3-layer MLP — no rank-8 matmuls on the PE.
  - weights and activations are bf16 on device (1 cycle/row on the PE, same
    as f32r, but half the DMA traffic and SBUF footprint); PSUM accumulation
    stays f32. Measured pipeline error ~4e-3 relative.
  - x is pre-transposed on the host so activations live as h^T
    [feat(part), batch(free)] with zero on-device transposes; the final
    layer uses h2^T as the *stationary* operand and w2 as the moving
    operand, producing natural-layout [batch, feat] output directly.
  - weights are pre-packed on the host into the exact SBUF tile layout so
    every DMA is >=2KB-contiguous per partition.
  - layer-2 bias arrives pre-broadcast [128, H3] and is added by the DVE
    while draining PSUM; layer-0/1 biases ride the activation instruction.
"""

import sys

if "/opt/trn_rl_repo" not in sys.path:
    sys.path.insert(0, "/opt/trn_rl_repo")

import numpy as np

T, B, D = 8, 1024, 1024
H1, H2, H3 = 2048, 2048, 1024
SCALING = 2.0  # alpha/rank = 16/8
P = 128
NT = 512  # PSUM free-dim tile (fp32 one-bank limit)

_CACHE = {}


def _build():
    import concourse.mybir as mybir
    from concourse import bacc
    from concourse.tile import TileContext
    from concourse.bass import ts

    f32 = mybir.dt.float32
    bf16 = mybir.dt.bfloat16
    AF = mybir.ActivationFunctionType

    nc = bacc.Bacc(None, target_bir_lowering=False, name="lora_mlp")

    KT0 = D // P      # 8  k-tiles, layer 0
    KT1 = H1 // P     # 16 k-tiles, layer 1
    KT2 = H2 // P     # 16 k-tiles, layer 2
    MT0 = H1 // P     # 16 m-tiles, layer 0
    MT1 = H2 // P     # 16 m-tiles, layer 1
    BT = B // P       # 8  batch 128-tiles
    NB = B // NT      # 2  batch 512-halves (free dim, layers 0/1)
    N2 = H3 // NT     # 2  feature 512-halves (free dim, layer 2)

    xt = nc.dram_tensor("xt", (D, B), bf16, kind="ExternalInput")
    w0 = nc.dram_tensor("w0", (MT0, P, KT0 * P), bf16, kind="ExternalInput")
    b0 = nc.dram_tensor("b0", (P, MT0), f32, kind="ExternalInput")
    w1 = nc.dram_tensor("w1", (MT1, P, KT1 * P), bf16, kind="ExternalInput")
    b1 = nc.dram_tensor("b1", (P, MT1), f32, kind="ExternalInput")
    w2 = nc.dram_tensor("w2", (H2, H3), bf16, kind="ExternalInput")
    b2 = nc.dram_tensor("b2", (P, H3), f32, kind="ExternalInput")
    out = nc.dram_tensor("out", (B, H3), f32, kind="ExternalOutput")

    with TileContext(nc) as tc:
        with (
            tc.tile_pool(name="main", bufs=1) as pool,
            tc.tile_pool(name="psum", bufs=1, space="PSUM") as pp,
        ):
            b0_sb = pool.tile([P, MT0], f32, tag="b0", bufs=1)
            nc.sync.dma_start(out=b0_sb, in_=b0[:, :])
            b1_sb = pool.tile([P, MT1], f32, tag="b1", bufs=1)
            nc.sync.dma_start(out=b1_sb, in_=b1[:, :])
            b2_sb = pool.tile([P, H3], f32, tag="b2", bufs=1)
            nc.scalar.dma_start(out=b2_sb, in_=b2[:, :])

            xT = []
            for k in range(KT0):
                xs = pool.tile([P, B], bf16, tag="X", bufs=KT0)
                xT.append(xs)
                nc.sync.dma_start(out=xs, in_=xt[ts(k, P), :])

            # =================== layer 0 ===================
            h0T = []
            for m in range(MT0):
                wt = pool.tile([P, KT0 * P], bf16, tag="W0", bufs=6)
                nc.sync.dma_start(out=wt, in_=w0[m])
                ht = pool.tile([P, B], bf16, tag="H0", bufs=MT0)
                h0T.append(ht)
                for n in range(NB):
                    ps = pp.tile([P, NT], f32, tag="pm", bufs=6)
                    for k in range(KT0):
                        nc.tensor.matmul(
                            ps,
                            wt[:, ts(k, P)],
                            xT[k][:, ts(n, NT)],
                            start=(k == 0),
                            stop=(k == KT0 - 1),
                        )
                    nc.scalar.activation(
                        ht[:, ts(n, NT)], ps, AF.Relu, bias=b0_sb[:, ts(m, 1)]
                    )

            # =================== layer 1 ===================
            h1T = []
            for m in range(MT1):
                wt = pool.tile([P, KT1 * P], bf16, tag="W1", bufs=6)
                nc.sync.dma_start(out=wt, in_=w1[m])
                ht = pool.tile([P, B], bf16, tag="H1", bufs=MT1)
                h1T.append(ht)
                for n in range(NB):
                    ps = pp.tile([P, NT], f32, tag="pm", bufs=6)
                    for k in range(KT1):
                        nc.tensor.matmul(
                            ps,
                            wt[:, ts(k, P)],
                            h0T[k][:, ts(n, NT)],
                            start=(k == 0),
                            stop=(k == KT1 - 1),
                        )
                    nc.scalar.activation(
                        ht[:, ts(n, NT)], ps, AF.Relu, bias=b1_sb[:, ts(m, 1)]
                    )

            # =================== layer 2 (natural output) ===================
            # w2 streams on the Activation-engine HWDGE queue so it never
            # queues behind the slot-paced w0/w1 stream on the sync queue.
            k2t = []
            for k in range(KT2):
                kt_ = pool.tile([P, H3], bf16, tag="K2", bufs=KT2)
                k2t.append(kt_)
                nc.scalar.dma_start(out=kt_, in_=w2[ts(k, P), :])
            for m in range(BT):
                ot = pool.tile([P, H3], f32, tag="O", bufs=4)
                for n in range(N2):
                    ps = pp.tile([P, NT], f32, tag="pm", bufs=6)
                    for k in range(KT2):
                        nc.tensor.matmul(
                            ps,
                            h1T[k][:, ts(m, P)],
                            k2t[k][:, ts(n, NT)],
                            start=(k == 0),
                            stop=(k == KT2 - 1),
                        )
                    nc.vector.tensor_add(ot[:, ts(n, NT)], ps, b2_sb[:, ts(n, NT)])
                nc.scalar.dma_start(out=out[ts(m, P), :], in_=ot)

    if not nc.is_finalized():
        nc.finalize()
    return nc


def _get_nc():
    if "nc" not in _CACHE:
        _CACHE["nc"] = _build()
    return _CACHE["nc"]


def _task_in_map(inputs, t, bf16, b0c, b1c, b2c):
    W0 = inputs["k0"] + SCALING * (inputs["d0"][:, :, t] @ inputs["u0"][:, :, t])
    W1 = inputs["k1"] + SCALING * (inputs["d1"][:, :, t] @ inputs["u1"][:, :, t])
    W2 = inputs["k2"] + SCALING * (inputs["d2"][:, :, t] @ inputs["u2"][:, :, t])
    # pack [K, M] -> [m, p, k*128+c] with element (m,p,kc) = W[k*128+p, m*128+c]
    w0r = np.ascontiguousarray(
        W0.reshape(8, 128, 16, 128).transpose(2, 1, 0, 3).reshape(16, 128, 1024),
        dtype=bf16,
    )
    w1r = np.ascontiguousarray(
        W1.reshape(16, 128, 16, 128).transpose(2, 1, 0, 3).reshape(16, 128, 2048),
        dtype=bf16,
    )
    w2r = np.ascontiguousarray(W2, dtype=bf16)
    xtr = np.ascontiguousarray(inputs["x"][t].T, dtype=bf16)
    return {
        "xt": xtr,
        "w0": w0r,
        "b0": b0c,
        "w1": w1r,
        "b1": b1c,
        "w2": w2r,
        "b2": b2c,
    }


def build_in_maps(inputs):
    import concurrent.futures

    import ml_dtypes

    bf16 = ml_dtypes.bfloat16
    b0c = np.ascontiguousarray(inputs["b0"].reshape(16, 128).T, dtype=np.float32)
    b1c = np.ascontiguousarray(inputs["b1"].reshape(16, 128).T, dtype=np.float32)
    b2c = np.ascontiguousarray(
        np.broadcast_to(inputs["b2"], (P, H3)), dtype=np.float32
    )
    with concurrent.futures.ThreadPoolExecutor(max_workers=T) as ex:
        in_maps = list(
            ex.map(lambda t: _task_in_map(inputs, t, bf16, b0c, b1c, b2c), range(T))
        )
    return in_maps


def kernel(**inputs):
    from concourse import bass_utils

    nc = _get_nc()
    in_maps = build_in_maps(inputs)
    res = bass_utils.run_bass_kernel_spmd(nc, in_maps, core_ids=list(range(T)))
    return np.stack([r["out"] for r in res.results], axis=0)
